# revision 14
# baseline (speedup 1.0000x reference)
"""Trainium2 Bass kernel for BinarySplitDecoder (binary-tree leaf probabilities).

Contract: kernel(x) takes the FULL input x [65536, 1023] fp32 and returns the
FULL output [65536, 1024] fp32 (leaf probabilities of a depth-10 binary split
tree, level-major node ordering).

Sharding: pure data parallel — batch dim split evenly across 8 NeuronCores.

Per-core kernel (rows_per_core = 8192, memory-bound at ~67 MB of HBM I/O):
  - Rows processed in chunks of G*128; partition p / free-group g holds batch
    row c*(128*G) + p*G + g, so every chunk DMA is one contiguous 2D block.
  - ScalarE computes oma = 1 - x per chunk (one ACT op, off the DVE).
  - DVE walks the tree level by level: left = cur * a ; right = cur * oma,
    written interleaved (stride 2) into the next level's tile. fp32
    tensor_tensor runs in 1x DVE mode regardless of stride, so the
    interleaved store is free. The final level's right-half runs on GPSIMD
    to shorten the DVE critical path; out-DMA streams from the same tile.
  - This matches the reference's fp32 operation sequence exactly
    (bitwise-identical output, no cancellation on small leaves).
"""

import numpy as np

import concourse.bacc as bacc
import concourse.bass as bass
import concourse.mybir as mybir
from concourse.tile import TileContext
from concourse.bass_utils import run_bass_kernel_spmd

TREE_DEPTH = 10
N_NODES = (1 << TREE_DEPTH) - 1  # 1023
N_LEAVES = 1 << TREE_DEPTH  # 1024
N_CORES = 8
P = 128  # SBUF partitions


def build_nc(
    rows_per_core: int,
    G: int = 4,
    oma_on_act: bool = True,
    right9_on_pool: bool = True,
) -> bass.Bass:
    """Build the per-core Bass program.

    rows_per_core must be divisible by G*128. The kernel reads DRAM input
    "x" [rows_per_core, 1023] and writes "y" [rows_per_core, 1024].
    """
    chunk_rows = G * P
    assert rows_per_core % chunk_rows == 0
    n_chunks = rows_per_core // chunk_rows
    f32 = mybir.dt.float32

    # Bacc (not raw Bass): Bacc.compile() runs generate_event_semaphores,
    # which splits multi-wait sync onto EventSemaphore instructions (TRN2
    # instructions have a single sync-wait slot).
    nc = bacc.Bacc("TRN2", target_bir_lowering=False, debug=False)
    x = nc.declare_dram_parameter("x", [rows_per_core, N_NODES], f32, isOutput=False)
    y = nc.declare_dram_parameter("y", [rows_per_core, N_LEAVES], f32, isOutput=True)

    # [chunk, partition, (group, nodes/leaves)] views of DRAM.
    xv = x.rearrange("(c p g) n -> c p (g n)", g=G, p=P)
    yv = y.rearrange("(c p g) m -> c p (g m)", g=G, p=P)

    with TileContext(nc) as tc:
        with (
            tc.tile_pool(name="xin", bufs=3) as xp,
            tc.tile_pool(name="oma", bufs=2) as omap,
            tc.tile_pool(name="out", bufs=2) as outp,
            # bufs=2: with one buffer, chunk c+1's level-0 write must wait
            # for GPSIMD's level-9 read of chunk c (WAR) — a per-chunk stall.
            tc.tile_pool(name="cur", bufs=2) as curp,
        ):
            if oma_on_act:
                # Pre-warm the ACT function table (the first ACTIVATE pays a
                # ~2.7us table load); overlaps with the first x DMA.
                warm = curp.tile([P, 1, 2], f32, tag="cur0")
                nc.gpsimd.memset(warm[:], 0.0)
                nc.scalar.activation(
                    out=warm[:],
                    in_=warm[:],
                    func=mybir.ActivationFunctionType.Copy,
                    bias=1.0,
                    scale=-1.0,
                )
            for c in range(n_chunks):
                xt = xp.tile([P, G, N_NODES], f32, tag="x")
                nc.sync.dma_start(out=xt[:], in_=xv[c])

                # oma = 1 - x for the whole chunk, one op off the DVE.
                oma_t = omap.tile([P, G, N_NODES], f32, tag="oma")
                if oma_on_act:
                    nc.scalar.activation(
                        out=oma_t[:],
                        in_=xt[:],
                        func=mybir.ActivationFunctionType.Copy,
                        bias=1.0,
                        scale=-1.0,
                    )
                else:
                    nc.vector.tensor_scalar(
                        out=oma_t[:],
                        in0=xt[:],
                        scalar1=-1.0,
                        scalar2=1.0,
                        op0=mybir.AluOpType.mult,
                        op1=mybir.AluOpType.add,
                    )

                out_t = outp.tile([P, G, N_LEAVES], f32, tag="y")
                cur = None
                for d in range(TREE_DEPTH):
                    L = 1 << d
                    if d == TREE_DEPTH - 1:
                        nxt = out_t
                    else:
                        # ping-pong intermediate levels between two shared
                        # slots (sized by the largest level using each tag)
                        nxt = curp.tile([P, G, 2 * L], f32, tag=f"cur{d % 2}")
                    a = xt[:, :, L - 1 : 2 * L - 1]  # [P, G, L] level-d alphas
                    oma = oma_t[:, :, L - 1 : 2 * L - 1]
                    left = nxt[:, :, 0::2]
                    right = nxt[:, :, 1::2]
                    if d == 0:
                        # cur == 1:  left = a, right = 1 - a
                        nc.vector.tensor_copy(out=left, in_=a)
                        nc.vector.tensor_copy(out=right, in_=oma)
                    else:
                        nc.vector.tensor_mul(out=left, in0=cur, in1=a)
                        eng = (
                            nc.gpsimd
                            if (right9_on_pool and d == TREE_DEPTH - 1)
                            else nc.vector
                        )
                        eng.tensor_mul(out=right, in0=cur, in1=oma)
                    cur = nxt

                nc.sync.dma_start(out=yv[c], in_=out_t[:])

    nc.compile()
    return nc


def _run(x: np.ndarray, **spmd_kwargs):
    """Shard x, run the Bass kernel on all 8 cores, return (y, BassKernelResults)."""
    x = np.ascontiguousarray(np.asarray(x, dtype=np.float32))
    B = x.shape[0]
    assert B % N_CORES == 0 and x.shape[1] == N_NODES
    rows_per_core = B // N_CORES

    nc = build_nc(rows_per_core)
    core_ids = list(range(N_CORES))
    in_maps = [
        {"x": x[i * rows_per_core : (i + 1) * rows_per_core]} for i in core_ids
    ]
    res = run_bass_kernel_spmd(nc, in_maps, core_ids, **spmd_kwargs)
    out = np.concatenate([r["y"] for r in res.results], axis=0)
    return out, res


def kernel(x: np.ndarray) -> np.ndarray:
    return _run(x)[0]


# revision 15
# speedup vs baseline: 1.1326x; 1.1326x over previous
"""Trainium2 Bass kernel for BinarySplitDecoder (binary-tree leaf probabilities).

Contract: kernel(x) takes the FULL input x [65536, 1023] fp32 and returns the
FULL output [65536, 1024] fp32 (leaf probabilities of a depth-10 binary split
tree, level-major node ordering).

Sharding: pure data parallel — batch dim split evenly across 8 NeuronCores.

Per-core kernel (rows_per_core = 8192, memory-bound at ~67 MB of HBM I/O):
  - Rows processed in chunks of G*128; partition p / free-group g holds batch
    row c*(128*G) + p*G + g, so every chunk DMA is one contiguous 2D block.
  - ScalarE computes oma = 1 - x per chunk (one ACT op, off the DVE).
  - DVE walks the tree level by level: left = cur * a ; right = cur * oma,
    written interleaved (stride 2) into the next level's tile. fp32
    tensor_tensor runs in 1x DVE mode regardless of stride, so the
    interleaved store is free. The final level's right-half runs on GPSIMD
    to shorten the DVE critical path; out-DMA streams from the same tile.
  - This matches the reference's fp32 operation sequence exactly
    (bitwise-identical output, no cancellation on small leaves).
"""

import numpy as np

import concourse.bacc as bacc
import concourse.bass as bass
import concourse.mybir as mybir
from concourse.tile import TileContext
from concourse.bass_utils import run_bass_kernel_spmd

TREE_DEPTH = 10
N_NODES = (1 << TREE_DEPTH) - 1  # 1023
N_LEAVES = 1 << TREE_DEPTH  # 1024
N_CORES = 8
P = 128  # SBUF partitions


def build_nc(
    rows_per_core: int,
    G: int = 4,
    oma_on_act: bool = True,
    # Concurrent GPSIMD tensor ops slow DVE ops by ~30% (SBUF port
    # contention, measured), so the level-9 offload to Pool is a net loss.
    right9_on_pool: bool = False,
) -> bass.Bass:
    """Build the per-core Bass program.

    rows_per_core must be divisible by G*128. The kernel reads DRAM input
    "x" [rows_per_core, 1023] and writes "y" [rows_per_core, 1024].
    """
    chunk_rows = G * P
    assert rows_per_core % chunk_rows == 0
    n_chunks = rows_per_core // chunk_rows
    f32 = mybir.dt.float32

    # Bacc (not raw Bass): Bacc.compile() runs generate_event_semaphores,
    # which splits multi-wait sync onto EventSemaphore instructions (TRN2
    # instructions have a single sync-wait slot).
    nc = bacc.Bacc("TRN2", target_bir_lowering=False, debug=False)
    x = nc.declare_dram_parameter("x", [rows_per_core, N_NODES], f32, isOutput=False)
    y = nc.declare_dram_parameter("y", [rows_per_core, N_LEAVES], f32, isOutput=True)

    # [chunk, partition, (group, nodes/leaves)] views of DRAM.
    xv = x.rearrange("(c p g) n -> c p (g n)", g=G, p=P)
    yv = y.rearrange("(c p g) m -> c p (g m)", g=G, p=P)

    with TileContext(nc) as tc:
        with (
            tc.tile_pool(name="xin", bufs=3) as xp,
            tc.tile_pool(name="oma", bufs=2) as omap,
            tc.tile_pool(name="out", bufs=2) as outp,
            # bufs=2: with one buffer, chunk c+1's level-0 write must wait
            # for GPSIMD's level-9 read of chunk c (WAR) — a per-chunk stall.
            tc.tile_pool(name="cur", bufs=2) as curp,
        ):
            if oma_on_act:
                # Pre-warm the ACT function table (the first ACTIVATE pays a
                # ~2.7us table load); overlaps with the first x DMA.
                warm = curp.tile([P, 1, 2], f32, tag="cur0")
                nc.gpsimd.memset(warm[:], 0.0)
                nc.scalar.activation(
                    out=warm[:],
                    in_=warm[:],
                    func=mybir.ActivationFunctionType.Copy,
                    bias=1.0,
                    scale=-1.0,
                )
            for c in range(n_chunks):
                xt = xp.tile([P, G, N_NODES], f32, tag="x")
                nc.sync.dma_start(out=xt[:], in_=xv[c])

                # oma = 1 - x for the whole chunk, one op off the DVE.
                oma_t = omap.tile([P, G, N_NODES], f32, tag="oma")
                if oma_on_act:
                    nc.scalar.activation(
                        out=oma_t[:],
                        in_=xt[:],
                        func=mybir.ActivationFunctionType.Copy,
                        bias=1.0,
                        scale=-1.0,
                    )
                else:
                    nc.vector.tensor_scalar(
                        out=oma_t[:],
                        in0=xt[:],
                        scalar1=-1.0,
                        scalar2=1.0,
                        op0=mybir.AluOpType.mult,
                        op1=mybir.AluOpType.add,
                    )

                out_t = outp.tile([P, G, N_LEAVES], f32, tag="y")
                cur = None
                for d in range(TREE_DEPTH):
                    L = 1 << d
                    if d == TREE_DEPTH - 1:
                        nxt = out_t
                    else:
                        # ping-pong intermediate levels between two shared
                        # slots (sized by the largest level using each tag)
                        nxt = curp.tile([P, G, 2 * L], f32, tag=f"cur{d % 2}")
                    a = xt[:, :, L - 1 : 2 * L - 1]  # [P, G, L] level-d alphas
                    oma = oma_t[:, :, L - 1 : 2 * L - 1]
                    left = nxt[:, :, 0::2]
                    right = nxt[:, :, 1::2]
                    if d == 0:
                        # cur == 1:  left = a, right = 1 - a
                        nc.vector.tensor_copy(out=left, in_=a)
                        nc.vector.tensor_copy(out=right, in_=oma)
                    else:
                        nc.vector.tensor_mul(out=left, in0=cur, in1=a)
                        eng = (
                            nc.gpsimd
                            if (right9_on_pool and d == TREE_DEPTH - 1)
                            else nc.vector
                        )
                        eng.tensor_mul(out=right, in0=cur, in1=oma)
                    cur = nxt

                nc.sync.dma_start(out=yv[c], in_=out_t[:])

    nc.compile()
    return nc


def _run(x: np.ndarray, **spmd_kwargs):
    """Shard x, run the Bass kernel on all 8 cores, return (y, BassKernelResults)."""
    x = np.ascontiguousarray(np.asarray(x, dtype=np.float32))
    B = x.shape[0]
    assert B % N_CORES == 0 and x.shape[1] == N_NODES
    rows_per_core = B // N_CORES

    nc = build_nc(rows_per_core)
    core_ids = list(range(N_CORES))
    in_maps = [
        {"x": x[i * rows_per_core : (i + 1) * rows_per_core]} for i in core_ids
    ]
    res = run_bass_kernel_spmd(nc, in_maps, core_ids, **spmd_kwargs)
    out = np.concatenate([r["y"] for r in res.results], axis=0)
    return out, res


def kernel(x: np.ndarray) -> np.ndarray:
    return _run(x)[0]
